# revision 28
# baseline (speedup 1.0000x reference)
"""Trainium2 Bass kernel for nn_MemoryBank_51135880626820 (scatter_memory).

Data-parallel over the query batch across 8 NeuronCores: the [32768, 256]
memory bank is replicated per core, each core handles 1024 query rows.

Per-core pipeline (v2):
  - phase A: normalize queries, PE-transpose to fp8 DoubleRow stationary layout
  - phase B (interleaved per 1024-row bank chunk): DMA chunk, fp32 norms,
    scale-cast to fp8, PE-transpose into resident d-major fp8 bank; then for
    each of 8 query tiles one fp8 DoubleRow matmul pair -> PSUM fp32 ->
    bf16 evacuation (scalar/gpsimd) -> DVE max8 + max_index screening
    (8 candidates per chunk, 256 per query total)
  - phase C per query tile: pack bf16 value + chunk-local index into fp32
    (exact), 3x max8/match_replace prune to top-24, recover chunk via
    max_index over the packed array, indirect-DMA gather of raw rows +
    reciprocal norms, exact fp32 re-dot (fused ttr/stt on DVE+gpsimd),
    top-8 + unnormalized softmax weights + weighted sum (renorm cancels the
    softmax denominator), rescale to ||q||.

self-contained: hardcodes all shapes; builds and caches the Bass program on
first call.
"""

import sys

for _p in ("/opt/trn_rl_repo",):
    if _p not in sys.path:
        sys.path.insert(0, _p)

import numpy as np

import concourse.bass as bass
import concourse.mybir as mybir
import concourse.tile as tile
from concourse.bass import IndirectOffsetOnAxis
from concourse.masks import make_identity

F32 = mybir.dt.float32
BF16 = mybir.dt.bfloat16
FP8 = mybir.dt.float8e4
U16 = mybir.dt.uint16
U32 = mybir.dt.uint32

N_CORES = 8
B = 8192
B_LOC = B // N_CORES        # 1024
M = 32768
D = 256
K = 8
NQT = B_LOC // 128          # 8 query tiles per core
CH = 1024                   # prep/psum sub-chunk (m columns)
NCH = M // CH               # 32 sub-chunks
SCH = 2048                  # screening window (max8/find8 width)
NSC = M // SCH              # 16 windows
NC = NSC * 8                # 128 candidates per query
LPX = 24                    # extraction width (3 rounds of max8)
LP = 20                     # pruned candidates (gathered + refined exactly)
NEG = -1.0e30
SELF_MATCH = 0.9999
EPACK = 2.0 ** -25          # chunk-local index packing epsilon (exact for |v|<0.5)

USE_DR = True               # fp8 DoubleRow matmul


# --------------------------------------------------------------------------
# workarounds for this container's walrus build, which rejects more than one
# sync-wait per instruction ("Too many sync wait commands").
# --------------------------------------------------------------------------
def _install_patches():
    import json

    import bass_rust
    import concourse.bass_utils as _bu
    import concourse.bass2jax as _b2j
    import concourse.tile as tile_mod
    from concourse.tile import TileContext

    if getattr(_bu, "_mb_patched", False):
        return

    try:
        ScopedClock = tile_mod.ScopedClock
    except AttributeError:
        ScopedClock = bass_rust.ScopedClock

    def _patched_drain_and_barrier(self, tick_clock, wait_clock):
        nc = self.nc
        drain_inst = nc.sync.drain()
        wait_clock.add_sem_waits(
            drain_inst.ins, ScopedClock({None: tick_clock.global_clock})
        )
        si = drain_inst.ins.sync_info
        waits = list(si.on_wait) if si is not None and si.on_wait else []
        if len(waits) > 1:
            drain_inst.ins.sync_info = bass_rust.SyncInfo(
                on_wait=[waits[0]],
                on_update=list(si.on_update) if si.on_update else [],
            )
            for w in waits[1:]:
                nop = nc.sync.nop(nofuse=True, hint="tail_wait")
                nop.ins.sync_info = bass_rust.SyncInfo(on_wait=[w], on_update=[])
        nc.all_engine_barrier()
        assert self.sems is not None
        popped = nc._tile_sem_poison_stack.pop()
        assert popped is self._sem_poison
        nc.clear_and_free_semaphores(list(self.sems.allocated().values()))
        nc.all_engine_barrier()

    TileContext._drain_and_barrier = _patched_drain_and_barrier

    def split_multiwaits(bir_json):
        m = json.loads(bir_json)
        changed = False
        for fn in m.get("functions", []):
            for bb in fn.get("blocks", []):
                insts = bb.get("instructions", [])
                out = []
                for ins in insts:
                    si = ins.get("sync_info") or {}
                    waits = si.get("on_wait") or []
                    if len(waits) > 1:
                        changed = True
                        for kk, w in enumerate(waits[:-1]):
                            out.append({
                                "debug": ins.get("debug", 0),
                                "engine": ins["engine"],
                                "ins": [],
                                "name": f"{ins['name']}-w{kk}",
                                "opcode": "NoOp",
                                "outs": [],
                                "sync_info": {"on_update": [], "on_wait": [w]},
                                "text_hint": "split_wait",
                            })
                        si = dict(si)
                        si["on_wait"] = [waits[-1]]
                        ins = dict(ins)
                        ins["sync_info"] = si
                    out.append(ins)
                bb["instructions"] = out
        if not changed:
            return bir_json
        return json.dumps(m).encode()

    _orig_compile = _bu.compile_bir_kernel

    def _patched_compile(bir_json, tmpdir, neff_name="file.neff"):
        if isinstance(bir_json, str):
            bir_json = bir_json.encode()
        return _orig_compile(split_multiwaits(bir_json), tmpdir, neff_name)

    _bu.compile_bir_kernel = _patched_compile
    _b2j.compile_bir_kernel = _patched_compile
    _bu._mb_patched = True


# --------------------------------------------------------------------------
# per-core Bass program
# --------------------------------------------------------------------------
def _build():
    nc = bass.Bass("TRN2", target_bir_lowering=False, debug=False)
    q_in = nc.dram_tensor("q", [B_LOC, D], F32, kind="ExternalInput")
    mem_in = nc.dram_tensor("mem", [M, D], F32, kind="ExternalInput")
    out = nc.dram_tensor("out", [B_LOC, D], F32, kind="ExternalOutput")

    AF = mybir.ActivationFunctionType
    AL = mybir.AluOpType
    DR = mybir.MatmulPerfMode.DoubleRow

    with tile.TileContext(nc) as tc, tc.tile_pool(name="resident", bufs=1) as res:
        with (
            tc.tile_pool(name="small", bufs=2) as small,
            tc.tile_pool(name="sec", bufs=3) as secp,
            tc.tile_pool(name="gat", bufs=2) as gat,
            tc.tile_pool(name="bigC", bufs=1) as bigc,
            tc.tile_pool(name="smallC", bufs=2) as sc,
            tc.tile_pool(name="psS", bufs=2, space="PSUM") as psS,
            tc.tile_pool(name="psT", bufs=2, space="PSUM") as psT,
        ):
            # ---- resident tiles ----
            nmT8 = res.tile([128, M, 2], FP8, tag="nmT8")       # bank, d-major fp8, ktile-interleaved
            qT8 = res.tile([128, NQT, 2, 128], FP8, tag="qT8")  # DR stationary per qtile
            nqf = res.tile([128, NQT, D], F32, tag="nqf")       # normalized queries fp32
            qnorm = res.tile([128, NQT], F32, tag="qnorm")
            cand_v = res.tile([128, NQT, NC], BF16, tag="cand_v")
            cand_iu = res.tile([128, NQT, NC], U16, tag="cand_iu")
            rnorm_sb = res.tile([128, NCH * 8], F32, tag="rnorm_sb")
            ident8 = res.tile([128, 128], BF16, tag="ident8")
            make_identity(nc, ident8[:])

            # ---- phase A: queries ----
            qn2 = res.tile([128, NQT], F32, tag="qn2")
            qrin = res.tile([128, NQT], F32, tag="qrin")
            for qt in range(NQT):
                qf = small.tile([128, D], F32, tag="qf")
                nc.sync.dma_start(qf[:], q_in[qt * 128:(qt + 1) * 128, :])
                gsqA = small.tile([128, D], F32, tag="sqs")
                nc.scalar.activation(gsqA[:], qf[:], AF.Square,
                                     accum_out=qn2[:, qt:qt + 1])
                nc.scalar.activation(qnorm[:, qt:qt + 1], qn2[:, qt:qt + 1], AF.Sqrt)
                nc.vector.reciprocal(qrin[:, qt:qt + 1], qnorm[:, qt:qt + 1])
                nc.scalar.activation(nqf[:, qt, :], qf[:], AF.Copy,
                                     scale=qrin[:, qt:qt + 1])
                nq8 = small.tile([128, D], BF16, tag="nq8")
                nc.vector.tensor_copy(nq8[:], nqf[:, qt, :])
                pq = psT.tile([128, 512], BF16, tag="psTt")
                for h in range(2):
                    nc.tensor.transpose(pq[:, h * 128:(h + 1) * 128],
                                        nq8[:, h * 128:(h + 1) * 128], ident8[:])
                nc.vector.tensor_copy(qT8[:, qt, :, :], pq[:, 0:256].rearrange("p (h j) -> p h j", h=2))

            # ---- per 2048-window: prep its two 1024 sub-chunks, then screen ----
            for w in range(NSC):
              for sub in range(2):
                c = w * 2 + sub
                mt = small.tile([128, 8, D], F32, tag="mt")
                nc.sync.dma_start(
                    mt[:],
                    mem_in[c * CH:(c + 1) * CH, :].rearrange("(g p) d -> p g d", p=128))
                n2 = small.tile([128, 8], F32, tag="n2")
                sqs = small.tile([128, D], F32, tag="sqs")
                for g in range(8):
                    nc.scalar.activation(sqs[:], mt[:, g, :], AF.Square,
                                         accum_out=n2[:, g:g + 1])
                nrm = small.tile([128, 8], F32, tag="nrm")
                nc.scalar.activation(nrm[:], n2[:], AF.Sqrt)
                nc.vector.reciprocal(rnorm_sb[:, c * 8:(c + 1) * 8], nrm[:])
                nm8c = small.tile([128, 8, D], BF16, tag="nm8c")
                nc.gpsimd.tensor_tensor(
                    out=nm8c[:], in0=mt[:],
                    in1=rnorm_sb[:, c * 8:(c + 1) * 8].rearrange("p (g o) -> p g o", o=1)
                        .to_broadcast([128, 8, D]),
                    op=AL.mult)
                for h in range(2):
                    pt = None
                    for g in range(8):
                        if g % 4 == 0:
                            pt = psT.tile([128, 512], BF16, tag="psTt")
                        nc.tensor.transpose(
                            pt[:, (g % 4) * 128:(g % 4 + 1) * 128],
                            nm8c[:, g, h * 128:(h + 1) * 128], ident8[:])
                        if g % 4 == 3:
                            nc.vector.tensor_copy(
                                nmT8[:, c * CH + (g - 3) * 128: c * CH + (g + 1) * 128, h],
                                pt[:])

              # screening for this window, 8 qtiles interleaved
              if True:
                for qt in range(NQT):
                    sec = secp.tile([128, SCH], BF16, tag="sec")
                    for sub in range(2):
                        m0 = w * SCH + sub * CH
                        ps = psS.tile([128, CH], F32, tag="ps")
                        for b2 in range(2):
                            nc.tensor.matmul(
                                ps[:, b2 * 512:(b2 + 1) * 512],
                                qT8[:, qt, :, :],
                                nmT8[:, m0 + b2 * 512: m0 + (b2 + 1) * 512, :]
                                    .rearrange("p m h -> p h m"),
                                start=True, stop=True, perf_mode=DR)
                        nc.scalar.copy(sec[:, sub * CH:(sub + 1) * CH], ps[:])
                    nc.vector.max(out=cand_v[:, qt, w * 8:(w + 1) * 8], in_=sec[:])
                    nc.vector.max_index(
                        out=cand_iu[:, qt, w * 8:(w + 1) * 8],
                        in_max=cand_v[:, qt, w * 8:(w + 1) * 8], in_values=sec[:])

            # ---- phase C: prune + gather + exact refine ----
            for qt in range(NQT):
                lif = sc.tile([128, NC], F32, tag="lif")
                nc.vector.tensor_copy(lif[:], cand_iu[:, qt, :])
                mskn = sc.tile([128, NC], F32, tag="mskn")
                nc.vector.tensor_scalar(
                    out=mskn[:], in0=cand_v[:, qt, :], scalar1=SELF_MATCH,
                    scalar2=NEG, op0=AL.is_ge, op1=AL.mult)
                packed = sc.tile([128, NC], F32, tag="packed")
                nc.vector.scalar_tensor_tensor(
                    out=packed[:], in0=lif[:], scalar=EPACK,
                    in1=cand_v[:, qt, :], op0=AL.mult, op1=AL.add)
                nc.vector.tensor_tensor(out=packed[:], in0=packed[:], in1=mskn[:], op=AL.add)

                pv = sc.tile([128, LPX], F32, tag="pv")
                posu = sc.tile([128, LPX], U16, tag="posu")
                packedB = sc.tile([128, NC], F32, tag="packedB")
                packedC = sc.tile([128, NC], F32, tag="packedC")
                nc.vector.max(out=pv[:, 0:8], in_=packed[:])
                nc.vector.max_index(out=posu[:, 0:8], in_max=pv[:, 0:8], in_values=packed[:])
                nc.vector.match_replace(out=packedB[:], in_to_replace=pv[:, 0:8],
                                        in_values=packed[:], imm_value=NEG)
                nc.vector.max(out=pv[:, 8:16], in_=packedB[:])
                nc.vector.max_index(out=posu[:, 8:16], in_max=pv[:, 8:16], in_values=packedB[:])
                nc.vector.match_replace(out=packedC[:], in_to_replace=pv[:, 8:16],
                                        in_values=packedB[:], imm_value=NEG)
                nc.vector.max(out=pv[:, 16:24], in_=packedC[:])
                nc.vector.max_index(out=posu[:, 16:24], in_max=pv[:, 16:24], in_values=packedC[:])

                # unpack: value (bf16 grid) + chunk-local idx; chunk from position
                vb = sc.tile([128, LPX], BF16, tag="vb")
                nc.vector.tensor_copy(vb[:], pv[:])
                vf = sc.tile([128, LPX], F32, tag="vf")
                nc.vector.tensor_copy(vf[:], vb[:])
                dd = sc.tile([128, LPX], F32, tag="dd")
                nc.vector.tensor_tensor(out=dd[:], in0=pv[:], in1=vf[:], op=AL.subtract)
                lidx = sc.tile([128, LPX], F32, tag="lidx")
                nc.vector.tensor_scalar(out=lidx[:], in0=dd[:], scalar1=float(2 ** 25),
                                        scalar2=None, op0=AL.mult)
                chk0 = sc.tile([128, LPX], U16, tag="chk0")
                nc.vector.tensor_scalar(out=chk0[:], in0=posu[:], scalar1=3,
                                        scalar2=None, op0=AL.logical_shift_right)
                chk = sc.tile([128, LPX], U16, tag="chk")
                nc.vector.tensor_scalar(out=chk[:], in0=chk0[:], scalar1=SCH,
                                        scalar2=None, op0=AL.mult)
                chkf = sc.tile([128, LPX], F32, tag="chkf")
                nc.vector.tensor_copy(chkf[:], chk[:])
                gidxf = sc.tile([128, LPX], F32, tag="gidxf")
                nc.vector.tensor_tensor(out=gidxf[:], in0=chkf[:], in1=lidx[:], op=AL.add)
                gidx = sc.tile([128, LPX], U32, tag="gidx")
                nc.vector.tensor_copy(gidx[:], gidxf[:])

                # gather raw rows (per-column indirect DMAs)
                G = gat.tile([128, LP, D], F32, tag="G")
                for cc in range(LP):
                    nc.gpsimd.indirect_dma_start(
                        out=G[:, cc, :], out_offset=None, in_=mem_in[:],
                        in_offset=IndirectOffsetOnAxis(ap=gidx[:, cc:cc + 1], axis=0))
                # row norms recomputed exactly on scalar (replaces an rnorm gather)
                gn2 = sc.tile([128, LP], F32, tag="gn2")
                gsq = sc.tile([128, D], F32, tag="gsq")
                for cc in range(LP):
                    nc.scalar.activation(gsq[:], G[:, cc, :], AF.Square,
                                         accum_out=gn2[:, cc:cc + 1])
                gn = sc.tile([128, LP], F32, tag="gn")
                nc.scalar.activation(gn[:], gn2[:], AF.Sqrt)
                rn = sc.tile([128, LP], F32, tag="rn")
                nc.vector.reciprocal(rn[:], gn[:])

                # exact re-dot: batched mult + reduce on DVE
                dots = sc.tile([128, LP], F32, tag="dots")
                scrD = bigc.tile([128, LP, D], F32, tag="scrD")
                nqb = nqf[:, qt, :].rearrange("p (o d) -> p o d", o=1).to_broadcast(
                    [128, LP, D])
                nc.vector.tensor_tensor(out=scrD[:], in0=G[:], in1=nqb, op=AL.mult)
                nc.vector.tensor_reduce(out=dots[:], in_=scrD[:],
                                        axis=mybir.AxisListType.X, op=AL.add)
                cos = sc.tile([128, LP], F32, tag="cos")
                nc.vector.tensor_tensor(out=cos[:], in0=dots[:], in1=rn[:], op=AL.mult)
                msk2 = sc.tile([128, LP], F32, tag="msk2")
                nc.vector.tensor_scalar(out=msk2[:], in0=cos[:], scalar1=SELF_MATCH,
                                        scalar2=NEG, op0=AL.is_ge, op1=AL.mult)
                nc.vector.tensor_tensor(out=cos[:], in0=cos[:], in1=msk2[:], op=AL.add)

                top8 = sc.tile([128, 8], F32, tag="top8")
                nc.vector.max(out=top8[:], in_=cos[:])
                wm = sc.tile([128, LP], F32, tag="wm")
                nc.vector.tensor_scalar(out=wm[:], in0=cos[:], scalar1=top8[:, 7:8],
                                        scalar2=None, op0=AL.is_ge)
                sh = sc.tile([128, LP], F32, tag="sh")
                nc.vector.tensor_scalar(out=sh[:], in0=cos[:], scalar1=top8[:, 0:1],
                                        scalar2=None, op0=AL.subtract)
                ex = sc.tile([128, LP], F32, tag="ex")
                nc.scalar.activation(ex[:], sh[:], AF.Exp)
                wts = sc.tile([128, LP], F32, tag="wts")
                nc.vector.tensor_tensor(out=wts[:], in0=ex[:], in1=wm[:], op=AL.mult)

                # weighted sum: batched weighted rows + strided cross-candidate reduce
                wb = wts[:].rearrange("p (c o) -> p c o", o=1).to_broadcast([128, LP, D])
                nc.vector.tensor_tensor(out=scrD[:], in0=G[:], in1=wb, op=AL.mult)
                accF = sc.tile([128, D], F32, tag="accF")
                nc.vector.tensor_reduce(out=accF[:], in_=scrD[:].rearrange("p c d -> p d c"),
                                        axis=mybir.AxisListType.X, op=AL.add)

                an2 = sc.tile([128, 1], F32, tag="an2")
                scrN = sc.tile([128, D], F32, tag="scrN")
                nc.vector.scalar_tensor_tensor(
                    out=scrN[:], in0=accF[:], scalar=1.0,
                    in1=accF[:], op0=AL.mult, op1=AL.mult, accum_out=an2[:])
                an = sc.tile([128, 1], F32, tag="an")
                nc.scalar.activation(an[:], an2[:], AF.Sqrt)
                ar = sc.tile([128, 1], F32, tag="ar")
                nc.vector.reciprocal(ar[:], an[:])
                scl = sc.tile([128, 1], F32, tag="scl")
                nc.vector.tensor_tensor(out=scl[:], in0=ar[:],
                                        in1=qnorm[:, qt:qt + 1], op=AL.mult)
                ot = sc.tile([128, D], F32, tag="ot")
                nc.scalar.activation(ot[:], accF[:], AF.Copy, scale=scl[:])
                nc.sync.dma_start(out[qt * 128:(qt + 1) * 128, :], ot[:])

    return nc


_CACHED_NC = None


def _get_nc():
    global _CACHED_NC
    if _CACHED_NC is None:
        _install_patches()
        _CACHED_NC = _build()
    return _CACHED_NC


def kernel(query, memory, k):
    query = np.ascontiguousarray(np.asarray(query, dtype=np.float32))
    memory = np.ascontiguousarray(np.asarray(memory, dtype=np.float32))
    k_val = int(np.asarray(k))
    assert query.shape == (B, D) and memory.shape == (M, D), (query.shape, memory.shape)
    assert k_val == K, f"kernel compiled for k={K}, got {k_val}"

    from concourse.bass_utils import run_bass_kernel_spmd

    nc = _get_nc()
    in_maps = [
        {"q": query[i * B_LOC: (i + 1) * B_LOC], "mem": memory}
        for i in range(N_CORES)
    ]
    res = run_bass_kernel_spmd(nc, in_maps, list(range(N_CORES)))
    return np.concatenate([res.results[i]["out"] for i in range(N_CORES)], axis=0)


# revision 29
# speedup vs baseline: 1.1894x; 1.1894x over previous
"""Trainium2 Bass kernel for nn_MemoryBank_51135880626820 (scatter_memory).

Data-parallel over the query batch across 8 NeuronCores: the [32768, 256]
memory bank is replicated per core, each core handles 1024 query rows.

Per-core pipeline (v2):
  - phase A: normalize queries, PE-transpose to fp8 DoubleRow stationary layout
  - phase B (interleaved per 1024-row bank chunk): DMA chunk, fp32 norms,
    scale-cast to fp8, PE-transpose into resident d-major fp8 bank; then for
    each of 8 query tiles one fp8 DoubleRow matmul pair -> PSUM fp32 ->
    bf16 evacuation (scalar/gpsimd) -> DVE max8 + max_index screening
    (8 candidates per chunk, 256 per query total)
  - phase C per query tile: pack bf16 value + chunk-local index into fp32
    (exact), 3x max8/match_replace prune to top-24, recover chunk via
    max_index over the packed array, indirect-DMA gather of raw rows +
    reciprocal norms, exact fp32 re-dot (fused ttr/stt on DVE+gpsimd),
    top-8 + unnormalized softmax weights + weighted sum (renorm cancels the
    softmax denominator), rescale to ||q||.

self-contained: hardcodes all shapes; builds and caches the Bass program on
first call.
"""

import sys

for _p in ("/opt/trn_rl_repo",):
    if _p not in sys.path:
        sys.path.insert(0, _p)

import numpy as np

import concourse.bass as bass
import concourse.mybir as mybir
import concourse.tile as tile
from concourse.bass import IndirectOffsetOnAxis
from concourse.masks import make_identity

F32 = mybir.dt.float32
BF16 = mybir.dt.bfloat16
FP8 = mybir.dt.float8e4
U16 = mybir.dt.uint16
U32 = mybir.dt.uint32

N_CORES = 8
B = 8192
B_LOC = B // N_CORES        # 1024
M = 32768
D = 256
K = 8
NQT = B_LOC // 128          # 8 query tiles per core
CH = 1024                   # prep/psum sub-chunk (m columns)
NCH = M // CH               # 32 sub-chunks
SCH = 2048                  # screening window (max8/find8 width)
NSC = M // SCH              # 16 windows
NC = NSC * 8                # 128 candidates per query
LPX = 24                    # extraction width (3 rounds of max8)
LP = 20                     # pruned candidates (gathered + refined exactly)
NEG = -1.0e30
SELF_MATCH = 0.9999
EPACK = 2.0 ** -25          # chunk-local index packing epsilon (exact for |v|<0.5)

USE_DR = True               # fp8 DoubleRow matmul


# --------------------------------------------------------------------------
# workarounds for this container's walrus build, which rejects more than one
# sync-wait per instruction ("Too many sync wait commands").
# --------------------------------------------------------------------------
def _install_patches():
    import json

    import bass_rust
    import concourse.bass_utils as _bu
    import concourse.bass2jax as _b2j
    import concourse.tile as tile_mod
    from concourse.tile import TileContext

    if getattr(_bu, "_mb_patched", False):
        return

    try:
        ScopedClock = tile_mod.ScopedClock
    except AttributeError:
        ScopedClock = bass_rust.ScopedClock

    def _patched_drain_and_barrier(self, tick_clock, wait_clock):
        nc = self.nc
        drain_inst = nc.sync.drain()
        wait_clock.add_sem_waits(
            drain_inst.ins, ScopedClock({None: tick_clock.global_clock})
        )
        si = drain_inst.ins.sync_info
        waits = list(si.on_wait) if si is not None and si.on_wait else []
        if len(waits) > 1:
            drain_inst.ins.sync_info = bass_rust.SyncInfo(
                on_wait=[waits[0]],
                on_update=list(si.on_update) if si.on_update else [],
            )
            for w in waits[1:]:
                nop = nc.sync.nop(nofuse=True, hint="tail_wait")
                nop.ins.sync_info = bass_rust.SyncInfo(on_wait=[w], on_update=[])
        nc.all_engine_barrier()
        assert self.sems is not None
        popped = nc._tile_sem_poison_stack.pop()
        assert popped is self._sem_poison
        nc.clear_and_free_semaphores(list(self.sems.allocated().values()))
        nc.all_engine_barrier()

    TileContext._drain_and_barrier = _patched_drain_and_barrier

    def split_multiwaits(bir_json):
        m = json.loads(bir_json)
        changed = False
        for fn in m.get("functions", []):
            for bb in fn.get("blocks", []):
                insts = bb.get("instructions", [])
                out = []
                for ins in insts:
                    si = ins.get("sync_info") or {}
                    waits = si.get("on_wait") or []
                    if len(waits) > 1:
                        changed = True
                        for kk, w in enumerate(waits[:-1]):
                            out.append({
                                "debug": ins.get("debug", 0),
                                "engine": ins["engine"],
                                "ins": [],
                                "name": f"{ins['name']}-w{kk}",
                                "opcode": "NoOp",
                                "outs": [],
                                "sync_info": {"on_update": [], "on_wait": [w]},
                                "text_hint": "split_wait",
                            })
                        si = dict(si)
                        si["on_wait"] = [waits[-1]]
                        ins = dict(ins)
                        ins["sync_info"] = si
                    out.append(ins)
                bb["instructions"] = out
        if not changed:
            return bir_json
        return json.dumps(m).encode()

    _orig_compile = _bu.compile_bir_kernel

    def _patched_compile(bir_json, tmpdir, neff_name="file.neff"):
        if isinstance(bir_json, str):
            bir_json = bir_json.encode()
        return _orig_compile(split_multiwaits(bir_json), tmpdir, neff_name)

    _bu.compile_bir_kernel = _patched_compile
    _b2j.compile_bir_kernel = _patched_compile
    _bu._mb_patched = True


# --------------------------------------------------------------------------
# per-core Bass program
# --------------------------------------------------------------------------
def _build():
    nc = bass.Bass("TRN2", target_bir_lowering=False, debug=False)
    q_in = nc.dram_tensor("q", [B_LOC, D], F32, kind="ExternalInput")
    mem_in = nc.dram_tensor("mem", [M, D], F32, kind="ExternalInput")
    out = nc.dram_tensor("out", [B_LOC, D], F32, kind="ExternalOutput")

    AF = mybir.ActivationFunctionType
    AL = mybir.AluOpType
    DR = mybir.MatmulPerfMode.DoubleRow

    with tile.TileContext(nc) as tc, tc.tile_pool(name="resident", bufs=1) as res:
        with (
            tc.tile_pool(name="small", bufs=2) as small,
            tc.tile_pool(name="sec", bufs=3) as secp,
            tc.tile_pool(name="psS", bufs=2, space="PSUM") as psS,
            tc.tile_pool(name="psT", bufs=2, space="PSUM") as psT,
        ):
            # ---- resident tiles ----
            nmT8 = res.tile([128, M, 2], FP8, tag="nmT8")       # bank, d-major fp8, ktile-interleaved
            qT8 = res.tile([128, NQT, 2, 128], FP8, tag="qT8")  # DR stationary per qtile
            nqf = res.tile([128, NQT, D], F32, tag="nqf")       # normalized queries fp32
            qnorm = res.tile([128, NQT], F32, tag="qnorm")
            cand_v = res.tile([128, NQT, NC], BF16, tag="cand_v")
            cand_iu = res.tile([128, NQT, NC], U16, tag="cand_iu")
            rnorm_sb = res.tile([128, NCH * 8], F32, tag="rnorm_sb")
            ident8 = res.tile([128, 128], BF16, tag="ident8")
            make_identity(nc, ident8[:])

            # ---- phase A: queries ----
            qn2 = res.tile([128, NQT], F32, tag="qn2")
            qrin = res.tile([128, NQT], F32, tag="qrin")
            for qt in range(NQT):
                qf = small.tile([128, D], F32, tag="qf")
                nc.sync.dma_start(qf[:], q_in[qt * 128:(qt + 1) * 128, :])
                gsqA = small.tile([128, D], F32, tag="sqs")
                nc.scalar.activation(gsqA[:], qf[:], AF.Square,
                                     accum_out=qn2[:, qt:qt + 1])
                nc.scalar.activation(qnorm[:, qt:qt + 1], qn2[:, qt:qt + 1], AF.Sqrt)
                nc.vector.reciprocal(qrin[:, qt:qt + 1], qnorm[:, qt:qt + 1])
                nc.scalar.activation(nqf[:, qt, :], qf[:], AF.Copy,
                                     scale=qrin[:, qt:qt + 1])
                nq8 = small.tile([128, D], BF16, tag="nq8")
                nc.vector.tensor_copy(nq8[:], nqf[:, qt, :])
                pq = psT.tile([128, 512], BF16, tag="psTt")
                for h in range(2):
                    nc.tensor.transpose(pq[:, h * 128:(h + 1) * 128],
                                        nq8[:, h * 128:(h + 1) * 128], ident8[:])
                nc.vector.tensor_copy(qT8[:, qt, :, :], pq[:, 0:256].rearrange("p (h j) -> p h j", h=2))

            # ---- per 2048-window: prep its two 1024 sub-chunks, then screen ----
            for w in range(NSC):
              for sub in range(2):
                c = w * 2 + sub
                mt = small.tile([128, 8, D], F32, tag="mt")
                nc.sync.dma_start(
                    mt[:],
                    mem_in[c * CH:(c + 1) * CH, :].rearrange("(g p) d -> p g d", p=128))
                n2 = small.tile([128, 8], F32, tag="n2")
                sqs = small.tile([128, D], F32, tag="sqs")
                for g in range(8):
                    nc.scalar.activation(sqs[:], mt[:, g, :], AF.Square,
                                         accum_out=n2[:, g:g + 1])
                nrm = small.tile([128, 8], F32, tag="nrm")
                nc.scalar.activation(nrm[:], n2[:], AF.Sqrt)
                nc.vector.reciprocal(rnorm_sb[:, c * 8:(c + 1) * 8], nrm[:])
                nm8c = small.tile([128, 8, D], BF16, tag="nm8c")
                nc.gpsimd.tensor_tensor(
                    out=nm8c[:], in0=mt[:],
                    in1=rnorm_sb[:, c * 8:(c + 1) * 8].rearrange("p (g o) -> p g o", o=1)
                        .to_broadcast([128, 8, D]),
                    op=AL.mult)
                for h in range(2):
                    pt = None
                    for g in range(8):
                        if g % 4 == 0:
                            pt = psT.tile([128, 512], BF16, tag="psTt")
                        nc.tensor.transpose(
                            pt[:, (g % 4) * 128:(g % 4 + 1) * 128],
                            nm8c[:, g, h * 128:(h + 1) * 128], ident8[:])
                        if g % 4 == 3:
                            nc.vector.tensor_copy(
                                nmT8[:, c * CH + (g - 3) * 128: c * CH + (g + 1) * 128, h],
                                pt[:])

              # screening for this window, 8 qtiles interleaved
              if True:
                for qt in range(NQT):
                    sec = secp.tile([128, SCH], BF16, tag="sec")
                    for sub in range(2):
                        m0 = w * SCH + sub * CH
                        ps = psS.tile([128, CH], F32, tag="ps")
                        for b2 in range(2):
                            nc.tensor.matmul(
                                ps[:, b2 * 512:(b2 + 1) * 512],
                                qT8[:, qt, :, :],
                                nmT8[:, m0 + b2 * 512: m0 + (b2 + 1) * 512, :]
                                    .rearrange("p m h -> p h m"),
                                start=True, stop=True, perf_mode=DR)
                        nc.scalar.copy(sec[:, sub * CH:(sub + 1) * CH], ps[:])
                    nc.vector.max(out=cand_v[:, qt, w * 8:(w + 1) * 8], in_=sec[:])
                    nc.vector.max_index(
                        out=cand_iu[:, qt, w * 8:(w + 1) * 8],
                        in_max=cand_v[:, qt, w * 8:(w + 1) * 8], in_values=sec[:])

        # ---- phase C: prune + gather + exact refine ----
        with (
            tc.tile_pool(name="gat", bufs=2) as gat,
            tc.tile_pool(name="bigC", bufs=1) as bigc,
            tc.tile_pool(name="smallC", bufs=2) as sc,
        ):
            for qt in range(NQT):
                lif = sc.tile([128, NC], F32, tag="lif")
                nc.vector.tensor_copy(lif[:], cand_iu[:, qt, :])
                mskn = sc.tile([128, NC], F32, tag="mskn")
                nc.vector.tensor_scalar(
                    out=mskn[:], in0=cand_v[:, qt, :], scalar1=SELF_MATCH,
                    scalar2=NEG, op0=AL.is_ge, op1=AL.mult)
                packed = sc.tile([128, NC], F32, tag="packed")
                nc.vector.scalar_tensor_tensor(
                    out=packed[:], in0=lif[:], scalar=EPACK,
                    in1=cand_v[:, qt, :], op0=AL.mult, op1=AL.add)
                nc.vector.tensor_tensor(out=packed[:], in0=packed[:], in1=mskn[:], op=AL.add)

                pv = sc.tile([128, LPX], F32, tag="pv")
                posu = sc.tile([128, LPX], U16, tag="posu")
                packedB = sc.tile([128, NC], F32, tag="packedB")
                packedC = sc.tile([128, NC], F32, tag="packedC")
                nc.vector.max(out=pv[:, 0:8], in_=packed[:])
                nc.vector.max_index(out=posu[:, 0:8], in_max=pv[:, 0:8], in_values=packed[:])
                nc.vector.match_replace(out=packedB[:], in_to_replace=pv[:, 0:8],
                                        in_values=packed[:], imm_value=NEG)
                nc.vector.max(out=pv[:, 8:16], in_=packedB[:])
                nc.vector.max_index(out=posu[:, 8:16], in_max=pv[:, 8:16], in_values=packedB[:])
                nc.vector.match_replace(out=packedC[:], in_to_replace=pv[:, 8:16],
                                        in_values=packedB[:], imm_value=NEG)
                nc.vector.max(out=pv[:, 16:24], in_=packedC[:])
                nc.vector.max_index(out=posu[:, 16:24], in_max=pv[:, 16:24], in_values=packedC[:])

                # unpack: value (bf16 grid) + chunk-local idx; chunk from position
                vb = sc.tile([128, LPX], BF16, tag="vb")
                nc.vector.tensor_copy(vb[:], pv[:])
                vf = sc.tile([128, LPX], F32, tag="vf")
                nc.vector.tensor_copy(vf[:], vb[:])
                dd = sc.tile([128, LPX], F32, tag="dd")
                nc.vector.tensor_tensor(out=dd[:], in0=pv[:], in1=vf[:], op=AL.subtract)
                lidx = sc.tile([128, LPX], F32, tag="lidx")
                nc.vector.tensor_scalar(out=lidx[:], in0=dd[:], scalar1=float(2 ** 25),
                                        scalar2=None, op0=AL.mult)
                chk0 = sc.tile([128, LPX], U16, tag="chk0")
                nc.vector.tensor_scalar(out=chk0[:], in0=posu[:], scalar1=3,
                                        scalar2=None, op0=AL.logical_shift_right)
                chk = sc.tile([128, LPX], U16, tag="chk")
                nc.vector.tensor_scalar(out=chk[:], in0=chk0[:], scalar1=SCH,
                                        scalar2=None, op0=AL.mult)
                chkf = sc.tile([128, LPX], F32, tag="chkf")
                nc.vector.tensor_copy(chkf[:], chk[:])
                gidxf = sc.tile([128, LPX], F32, tag="gidxf")
                nc.vector.tensor_tensor(out=gidxf[:], in0=chkf[:], in1=lidx[:], op=AL.add)
                gidx = sc.tile([128, LPX], U32, tag="gidx")
                nc.vector.tensor_copy(gidx[:], gidxf[:])

                # gather raw rows (per-column indirect DMAs)
                G = gat.tile([128, LP, D], F32, tag="G")
                for cc in range(LP):
                    nc.gpsimd.indirect_dma_start(
                        out=G[:, cc, :], out_offset=None, in_=mem_in[:],
                        in_offset=IndirectOffsetOnAxis(ap=gidx[:, cc:cc + 1], axis=0))
                # row norms recomputed exactly on scalar (replaces an rnorm gather)
                gn2 = sc.tile([128, LP], F32, tag="gn2")
                gsq = sc.tile([128, D], F32, tag="gsq")
                for cc in range(LP):
                    nc.scalar.activation(gsq[:], G[:, cc, :], AF.Square,
                                         accum_out=gn2[:, cc:cc + 1])
                gn = sc.tile([128, LP], F32, tag="gn")
                nc.scalar.activation(gn[:], gn2[:], AF.Sqrt)
                rn = sc.tile([128, LP], F32, tag="rn")
                nc.vector.reciprocal(rn[:], gn[:])

                # exact re-dot: batched mult + reduce on DVE
                dots = sc.tile([128, LP], F32, tag="dots")
                scrD = bigc.tile([128, LP, D], F32, tag="scrD")
                nqb = nqf[:, qt, :].rearrange("p (o d) -> p o d", o=1).to_broadcast(
                    [128, LP, D])
                nc.vector.tensor_tensor(out=scrD[:], in0=G[:], in1=nqb, op=AL.mult)
                nc.vector.tensor_reduce(out=dots[:], in_=scrD[:],
                                        axis=mybir.AxisListType.X, op=AL.add)
                cos = sc.tile([128, LP], F32, tag="cos")
                nc.vector.tensor_tensor(out=cos[:], in0=dots[:], in1=rn[:], op=AL.mult)
                msk2 = sc.tile([128, LP], F32, tag="msk2")
                nc.vector.tensor_scalar(out=msk2[:], in0=cos[:], scalar1=SELF_MATCH,
                                        scalar2=NEG, op0=AL.is_ge, op1=AL.mult)
                nc.vector.tensor_tensor(out=cos[:], in0=cos[:], in1=msk2[:], op=AL.add)

                top8 = sc.tile([128, 8], F32, tag="top8")
                nc.vector.max(out=top8[:], in_=cos[:])
                wm = sc.tile([128, LP], F32, tag="wm")
                nc.vector.tensor_scalar(out=wm[:], in0=cos[:], scalar1=top8[:, 7:8],
                                        scalar2=None, op0=AL.is_ge)
                sh = sc.tile([128, LP], F32, tag="sh")
                nc.vector.tensor_scalar(out=sh[:], in0=cos[:], scalar1=top8[:, 0:1],
                                        scalar2=None, op0=AL.subtract)
                ex = sc.tile([128, LP], F32, tag="ex")
                nc.scalar.activation(ex[:], sh[:], AF.Exp)
                wts = sc.tile([128, LP], F32, tag="wts")
                nc.vector.tensor_tensor(out=wts[:], in0=ex[:], in1=wm[:], op=AL.mult)

                # weighted sum: batched weighted rows + strided cross-candidate reduce
                wb = wts[:].rearrange("p (c o) -> p c o", o=1).to_broadcast([128, LP, D])
                nc.vector.tensor_tensor(out=scrD[:], in0=G[:], in1=wb, op=AL.mult)
                accF = sc.tile([128, D], F32, tag="accF")
                nc.vector.tensor_reduce(out=accF[:], in_=scrD[:].rearrange("p c d -> p d c"),
                                        axis=mybir.AxisListType.X, op=AL.add)

                an2 = sc.tile([128, 1], F32, tag="an2")
                scrN = sc.tile([128, D], F32, tag="scrN")
                nc.vector.scalar_tensor_tensor(
                    out=scrN[:], in0=accF[:], scalar=1.0,
                    in1=accF[:], op0=AL.mult, op1=AL.mult, accum_out=an2[:])
                an = sc.tile([128, 1], F32, tag="an")
                nc.scalar.activation(an[:], an2[:], AF.Sqrt)
                ar = sc.tile([128, 1], F32, tag="ar")
                nc.vector.reciprocal(ar[:], an[:])
                scl = sc.tile([128, 1], F32, tag="scl")
                nc.vector.tensor_tensor(out=scl[:], in0=ar[:],
                                        in1=qnorm[:, qt:qt + 1], op=AL.mult)
                ot = sc.tile([128, D], F32, tag="ot")
                nc.scalar.activation(ot[:], accF[:], AF.Copy, scale=scl[:])
                nc.sync.dma_start(out[qt * 128:(qt + 1) * 128, :], ot[:])

    return nc


_CACHED_NC = None


def _get_nc():
    global _CACHED_NC
    if _CACHED_NC is None:
        _install_patches()
        _CACHED_NC = _build()
    return _CACHED_NC


def kernel(query, memory, k):
    query = np.ascontiguousarray(np.asarray(query, dtype=np.float32))
    memory = np.ascontiguousarray(np.asarray(memory, dtype=np.float32))
    k_val = int(np.asarray(k))
    assert query.shape == (B, D) and memory.shape == (M, D), (query.shape, memory.shape)
    assert k_val == K, f"kernel compiled for k={K}, got {k_val}"

    from concourse.bass_utils import run_bass_kernel_spmd

    nc = _get_nc()
    in_maps = [
        {"q": query[i * B_LOC: (i + 1) * B_LOC], "mem": memory}
        for i in range(N_CORES)
    ]
    res = run_bass_kernel_spmd(nc, in_maps, list(range(N_CORES)))
    return np.concatenate([res.results[i]["out"] for i in range(N_CORES)], axis=0)
